# revision 5
# baseline (speedup 1.0000x reference)
"""Bass/Trainium2 kernel for a binarized NN (BNN) forward pass, data-parallel
over 8 NeuronCores.

Reference semantics (fp32):
    h1 = x @ sign(W1).T;  b1 = sign(h1 - mean(h1, axis=0))        # g=1, b=0
    h2 = b1 @ sign(W2).T; v2 = noisy_sign(h2, u2); b2 = v2         # BN+sign is
    h3 = b2 @ sign(W3).T; v3 = noisy_sign(h3, u3); b3 = v3         # identity on +-1
    out = b3 @ sign(W4).T

Math facts exploited (exactness):
  * b in {+-1,0} and sign(W) in {+-1} make h2/h3/out exact small integers under
    fp32 accumulation in any order -> bf16 matmuls on PE are bit-exact.
  * batchnorm+sign on +-1 inputs is the identity (|mean| < 1), so no batch
    statistics and no cross-core communication are needed for layers 2/3.
  * mean(h1, axis=0) == mean(x, axis=0) @ sign(W1).T -> computed on host in
    float64 (tiny dot), passed in as a per-feature threshold.
  * The stochastic flip (u < 0.5*exp(-h^2/50)) & (|h| <= 50) with h an exact
    even integer depends only on |h| in {0,2,...,50}: precompute on host
    A(u) = smallest even a with p(a) <= u, then flip <=> |h| < A. Exact.

Per-core layout is feature-major ("transposed"): activations live as
[features(partitions), batch(free)], so batch stays on the free dim and no
on-device transposes are needed. Batch 16384 is sharded 2048/core.
"""

from contextlib import ExitStack

import numpy as np

import concourse.bass as bass  # noqa: F401  (kept for AP helpers)
import concourse.tile as tile
from concourse import bacc, mybir
from concourse.bass_utils import run_bass_kernel_spmd

F32 = mybir.dt.float32
BF16 = mybir.dt.bfloat16
ALU = mybir.AluOpType
ACTF = mybir.ActivationFunctionType

N_CORES = 8
B = 16384                 # full batch
BC = B // N_CORES         # batch per core
HB = BC // 2              # half-batch processed per elementwise slice
D_IN = 784                # layer-1 input features
D_H = 1024                # hidden features
D_OUT = 10                # output features
K1 = (D_IN + 127) // 128  # 7 k-chunks for layer 1 (6 full + 16 rows)
K1_LAST = D_IN - 128 * (K1 - 1)
KH = D_H // 128           # 8 k-chunks for hidden layers
OC = D_H // 128           # 8 output-feature chunks

# float32(0.5*exp(-(a*a)/50)) for a = 0,2,...,50, computed with jnp.exp on the
# same jax backend the reference uses (fallback if jax is unavailable here).
_PTABLE_BITS = [
    0x3F000000, 0x3EEC515A, 0x3EB9E4E3, 0x3E79375C, 0x3E0E5ACB, 0x3D8A9501,
    0x3CE5ED93, 0x3C2289CB, 0x3B43D285, 0x3A4909DD, 0x392FE09E, 0x38031DFC,
    0x36A696B8, 0x35345CD8, 0x33A6674D, 0x3202D2C5, 0x302F4A31, 0x2E4824C7,
    0x2C42BB52, 0x2A2173E9, 0x27E4229E, 0x258959AD, 0x230CEE5E, 0x207672F6,
    0x1DB79FE2, 0x1AE92B5E,
]


def _prob_table() -> np.ndarray:
    """p(a) for a = 0,2,...,50, bit-matching the reference's jnp.exp."""
    try:
        import jax.numpy as jnp

        a = np.arange(0, 51, 2, dtype=np.float32)
        p = np.asarray(0.5 * jnp.exp(-(jnp.asarray(a) * a) / (2.0 * 5.0**2)),
                       dtype=np.float32)
        if p.shape == (26,) and np.all(np.diff(p) < 0):
            return p
    except Exception:
        pass
    return np.array(_PTABLE_BITS, dtype=np.uint32).view(np.float32)


def _flip_thresholds(u: np.ndarray, ptable: np.ndarray) -> np.ndarray:
    """A(u): flip <=> |h| < A. A = 52 - 2 * #{a : p(a) <= u}."""
    tab = ptable[::-1].copy()  # ascending: p(50), p(48), ..., p(0)
    idx = np.searchsorted(tab, u, side="right")
    return (52 - 2 * idx).astype(np.float32)




def emit_noisy_sign(nc, pool, ps, a_ap, out_ap):
    """out = noisy sign of h (exact): +1 iff (h>0)+(h<A)+(-h<A) == 2.

    Covers flip semantics including h==0 (s=-1) and the |h|<=50 cutoff
    (A <= 52), using only walrus-supported DVE op combos.
    """
    hb = ps.shape[-1]
    e1 = pool.tile([128, hb], F32, tag="e1")
    e2 = pool.tile([128, hb], F32, tag="e2")
    e3 = pool.tile([128, hb], F32, tag="e3")
    nc.vector.tensor_scalar(e1[:], ps[:], 0.0, None, op0=ALU.is_gt)   # h > 0
    nc.vector.tensor_tensor(e2[:], ps[:], a_ap, op=ALU.is_lt)         # h < A
    nc.vector.scalar_tensor_tensor(e3[:], ps[:], -1.0, a_ap,
                                   op0=ALU.mult, op1=ALU.is_lt)       # -h < A
    c23 = pool.tile([128, hb], F32, tag="c23")
    nc.vector.tensor_tensor(c23[:], e2[:], e3[:], op=ALU.add)
    g01 = pool.tile([128, hb], F32, tag="g01")
    # flip = (|h| < A) = (c23 == 2); g = flip XOR (h > 0)
    nc.vector.scalar_tensor_tensor(g01[:], c23[:], 2.0, e1[:],
                                   op0=ALU.is_equal, op1=ALU.not_equal)
    nc.scalar.activation(out_ap, g01[:], ACTF.Copy, bias=-1.0, scale=2.0)

def build_nc(repeat: int = 1):
    """Build the per-core Bass program (same program on all 8 cores)."""
    nc = bacc.Bacc("TRN2", target_bir_lowering=False, debug=False,
                   num_devices=N_CORES)

    xt = nc.dram_tensor("xt", [D_IN, BC], F32, kind="ExternalInput").ap()
    a2 = nc.dram_tensor("a2", [D_H, BC], F32, kind="ExternalInput").ap()
    a3 = nc.dram_tensor("a3", [D_H, BC], F32, kind="ExternalInput").ap()
    w1 = nc.dram_tensor("w1", [D_IN, D_H], F32, kind="ExternalInput").ap()
    w2 = nc.dram_tensor("w2", [D_H, D_H], BF16, kind="ExternalInput").ap()
    w3 = nc.dram_tensor("w3", [D_H, D_H], BF16, kind="ExternalInput").ap()
    w4 = nc.dram_tensor("w4", [D_H, D_OUT], BF16, kind="ExternalInput").ap()
    c1 = nc.dram_tensor("c1", [128, OC], F32, kind="ExternalInput").ap()
    out = nc.dram_tensor("out", [D_OUT, BC], F32, kind="ExternalOutput").ap()

    with tile.TileContext(nc) as tc:
        with ExitStack() as ctx:
            consts = ctx.enter_context(tc.tile_pool(name="consts", bufs=1))
            panels = ctx.enter_context(tc.tile_pool(name="panels", bufs=1))

            # Replicated weights + layer-1 thresholds.
            w2_t = consts.tile([128, KH, D_H], BF16, tag="w2")
            w3_t = consts.tile([128, KH, D_H], BF16, tag="w3")
            w4_t = consts.tile([128, KH, D_OUT], BF16, tag="w4")
            c1_t = consts.tile([128, OC], F32, tag="c1")
            w1_t = consts.tile([128, K1, D_H], F32, tag="w1")
            for k in range(KH):
                nc.gpsimd.dma_start(w2_t[:, k], w2[k * 128:(k + 1) * 128, :])
                nc.gpsimd.dma_start(w3_t[:, k], w3[k * 128:(k + 1) * 128, :])
                nc.gpsimd.dma_start(w4_t[:, k], w4[k * 128:(k + 1) * 128, :])
            nc.gpsimd.dma_start(c1_t[:], c1[:, :])
            for k in range(K1 - 1):
                nc.gpsimd.dma_start(w1_t[:, k], w1[k * 128:(k + 1) * 128, :])
            nc.gpsimd.dma_start(w1_t[:K1_LAST, K1 - 1],
                                w1[128 * (K1 - 1):D_IN, :])

            # +-1 activation panels, feature-major bf16 (b3 is fused away).
            b1_t = panels.tile([128, OC, BC], BF16, tag="b1")
            b2_t = panels.tile([128, OC, BC], BF16, tag="b2")

            for _rep in range(repeat):
                # ---- Layer 1: fp32 matmul + threshold sign -> b1 ----
                with ExitStack() as l1s:
                    l1x = l1s.enter_context(tc.tile_pool(name="l1x", bufs=2))
                    l1psum = l1s.enter_context(
                        tc.tile_pool(name="l1psum", bufs=2, space="PSUM"))
                    l1tmp = l1s.enter_context(tc.tile_pool(name="l1tmp", bufs=2))

                    for half in range(2):
                        hsl = slice(half * HB, (half + 1) * HB)
                        xt_t = l1x.tile([128, K1, HB], F32, tag="xt")
                        for k in range(K1 - 1):
                            nc.gpsimd.dma_start(
                                xt_t[:, k], xt[k * 128:(k + 1) * 128, hsl])
                        nc.gpsimd.dma_start(xt_t[:K1_LAST, K1 - 1],
                                            xt[128 * (K1 - 1):D_IN, hsl])

                        for o in range(OC):
                            ps = l1psum.tile([128, HB], F32, tag="mm")
                            for k in range(K1):
                                kk = K1_LAST if k == K1 - 1 else 128
                                lhsT = w1_t[:kk, k, o * 128:(o + 1) * 128]
                                for n in range(HB // 512):
                                    nc.tensor.matmul(
                                        ps[:, n * 512:(n + 1) * 512],
                                        lhsT,
                                        xt_t[:kk, k, n * 512:(n + 1) * 512],
                                        start=(k == 0),
                                        stop=(k == K1 - 1),
                                    )
                            # b1 = sign(h1 - c1): (h>c) - (h<c) in {-1, 0, +1}
                            gt = l1tmp.tile([128, HB], F32, tag="gt")
                            lt = l1tmp.tile([128, HB], F32, tag="lt")
                            cs = c1_t[:, o:o + 1]
                            nc.vector.tensor_scalar(gt[:], ps[:], cs, None,
                                                    op0=ALU.is_gt)
                            nc.vector.tensor_scalar(lt[:], ps[:], cs, None,
                                                    op0=ALU.is_lt)
                            nc.vector.tensor_tensor(b1_t[:, o, hsl], gt[:],
                                                    lt[:], op=ALU.subtract)

                # ---- Layer 2: bf16 integer matmul + exact noisy sign ----
                with ExitStack() as l2s:
                    lpsum = l2s.enter_context(
                        tc.tile_pool(name="l2psum", bufs=2, space="PSUM"))
                    ltmp = l2s.enter_context(tc.tile_pool(name="l2tmp", bufs=2))
                    lap = l2s.enter_context(tc.tile_pool(name="l2a", bufs=2))
                    for o in range(OC):
                        a_t = lap.tile([128, BC], F32, tag="a")
                        nc.gpsimd.dma_start(a_t[:],
                                            a2[o * 128:(o + 1) * 128, :])
                        for half in range(2):
                            hsl = slice(half * HB, (half + 1) * HB)
                            ps = lpsum.tile([128, HB], F32, tag="mm")
                            for k in range(KH):
                                lhsT = w2_t[:, k, o * 128:(o + 1) * 128]
                                for n in range(HB // 512):
                                    nc.tensor.matmul(
                                        ps[:, n * 512:(n + 1) * 512],
                                        lhsT,
                                        b1_t[:, k,
                                             half * HB + n * 512:
                                             half * HB + (n + 1) * 512],
                                        start=(k == 0),
                                        stop=(k == KH - 1),
                                    )
                            emit_noisy_sign(nc, ltmp, ps, a_t[:, hsl],
                                            b2_t[:, o, hsl])

                # ---- Layer 3 + fused layer 4 ----
                with ExitStack() as l3s:
                    l4psum = l3s.enter_context(
                        tc.tile_pool(name="l4psum", bufs=1, space="PSUM"))
                    l4out = l3s.enter_context(tc.tile_pool(name="l4out", bufs=1))
                    lpsum = l3s.enter_context(
                        tc.tile_pool(name="l3psum", bufs=2, space="PSUM"))
                    ltmp = l3s.enter_context(tc.tile_pool(name="l3tmp", bufs=2))
                    lap = l3s.enter_context(tc.tile_pool(name="l3a", bufs=2))
                    lb3 = l3s.enter_context(tc.tile_pool(name="l3b3", bufs=2))

                    ops = l4psum.tile([D_OUT, BC], F32, tag="out")
                    for o in range(OC):
                        a_t = lap.tile([128, BC], F32, tag="a")
                        nc.gpsimd.dma_start(a_t[:],
                                            a3[o * 128:(o + 1) * 128, :])
                        b3c = lb3.tile([128, BC], BF16, tag="b3c")
                        for half in range(2):
                            hsl = slice(half * HB, (half + 1) * HB)
                            ps = lpsum.tile([128, HB], F32, tag="mm")
                            for k in range(KH):
                                lhsT = w3_t[:, k, o * 128:(o + 1) * 128]
                                for n in range(HB // 512):
                                    nc.tensor.matmul(
                                        ps[:, n * 512:(n + 1) * 512],
                                        lhsT,
                                        b2_t[:, k,
                                             half * HB + n * 512:
                                             half * HB + (n + 1) * 512],
                                        start=(k == 0),
                                        stop=(k == KH - 1),
                                    )
                            emit_noisy_sign(nc, ltmp, ps, a_t[:, hsl],
                                            b3c[:, hsl])
                        # fused layer 4: accumulate out += w4[o].T @ b3c
                        for n in range(BC // 512):
                            nc.tensor.matmul(
                                ops[:, n * 512:(n + 1) * 512],
                                w4_t[:, o, :],
                                b3c[:, n * 512:(n + 1) * 512],
                                start=(o == 0),
                                stop=(o == OC - 1),
                            )
                    ot = l4out.tile([D_OUT, BC], F32, tag="ot")
                    nc.scalar.activation(ot[:], ops[:], ACTF.Copy)
                    nc.gpsimd.dma_start(out[:, :], ot[:])

    nc.compile()
    return nc


_NC_CACHE: dict[int, object] = {}


def _get_nc(repeat: int = 1):
    if repeat not in _NC_CACHE:
        _NC_CACHE[repeat] = build_nc(repeat)
    return _NC_CACHE[repeat]


def make_in_maps(x, u2, u3, W1, W2, W3, W4, **_unused):
    """Host preprocessing -> per-core input dicts."""
    import ml_dtypes

    x = np.ascontiguousarray(np.asarray(x, dtype=np.float32))
    W1b = np.sign(np.asarray(W1, dtype=np.float32))
    # mean(h1, axis=0) = sign(W1) @ mean(x, axis=0), in float64.
    mu1 = (W1b.astype(np.float64) @ x.mean(axis=0, dtype=np.float64)).astype(
        np.float32)
    c1 = np.ascontiguousarray(mu1.reshape(OC, 128).T)  # [128, OC]

    pt = _prob_table()
    a2f = _flip_thresholds(np.asarray(u2), pt)
    a3f = _flip_thresholds(np.asarray(u3), pt)

    xt = np.ascontiguousarray(x.T)                       # [784, B]
    a2t = np.ascontiguousarray(a2f.T)                    # [1024, B]
    a3t = np.ascontiguousarray(a3f.T)
    w1t = np.ascontiguousarray(W1b.T)                    # [784, 1024] fp32
    w2t = np.ascontiguousarray(
        np.sign(np.asarray(W2, np.float32)).T).astype(ml_dtypes.bfloat16)
    w3t = np.ascontiguousarray(
        np.sign(np.asarray(W3, np.float32)).T).astype(ml_dtypes.bfloat16)
    w4t = np.ascontiguousarray(
        np.sign(np.asarray(W4, np.float32)).T).astype(ml_dtypes.bfloat16)

    in_maps = []
    for c in range(N_CORES):
        sl = slice(c * BC, (c + 1) * BC)
        in_maps.append({
            "xt": np.ascontiguousarray(xt[:, sl]),
            "a2": np.ascontiguousarray(a2t[:, sl]),
            "a3": np.ascontiguousarray(a3t[:, sl]),
            "w1": w1t,
            "w2": w2t,
            "w3": w3t,
            "w4": w4t,
            "c1": c1,
        })
    return in_maps


def kernel(x, u2, u3, W1, W2, W3, W4,
           g1=None, b1=None, g2=None, b2=None, g3=None, b3=None):
    for g in (g1, g2, g3):
        assert g is None or np.all(np.asarray(g) > 0), "kernel assumes g > 0"
    for b in (b1, b2, b3):
        assert b is None or np.all(np.asarray(b) == 0), "kernel assumes b == 0"

    nc = _get_nc(repeat=1)
    in_maps = make_in_maps(x, u2, u3, W1, W2, W3, W4)
    res = run_bass_kernel_spmd(nc, in_maps, core_ids=list(range(N_CORES)))

    out = np.empty((B, D_OUT), dtype=np.float32)
    for c in range(N_CORES):
        out[c * BC:(c + 1) * BC, :] = res.results[c]["out"].T
    return out


# revision 18
# speedup vs baseline: 555.3361x; 555.3361x over previous
"""Bass/Trainium2 kernel for a binarized NN (BNN) forward pass, data-parallel
over 8 NeuronCores.

Reference semantics (fp32):
    h1 = x @ sign(W1).T;  b1 = sign(h1 - mean(h1, axis=0))        # g=1, b=0
    h2 = b1 @ sign(W2).T; v2 = noisy_sign(h2, u2); b2 = v2         # BN+sign is
    h3 = b2 @ sign(W3).T; v3 = noisy_sign(h3, u3); b3 = v3         # identity on +-1
    out = b3 @ sign(W4).T

Math facts exploited (exactness):
  * b in {+-1,0} and sign(W) in {+-1} make h2/h3/out exact small integers under
    fp32 PSUM accumulation in any order -> fp8 (e4m3) matmuls on PE are
    bit-exact, enabling DoubleRow perf mode (2 contraction rows/cycle).
  * batchnorm+sign on +-1 inputs is the identity (|mean| < 1), so no batch
    statistics and no cross-core communication are needed for layers 2/3.
  * mean(h1, axis=0) == mean(x, axis=0) @ sign(W1).T -> computed on host in
    float64 (tiny dot), passed in as a per-feature threshold c1. Layer 1 runs
    as a true fp32 PE matmul (same fp32 pathway XLA uses for the reference),
    which measured bit-identical sign decisions vs the reference.
  * The stochastic flip (u < 0.5*exp(-h^2/50)) & (|h| <= 50) with h an exact
    even integer depends only on |h| in {0,2,...,50}: precompute on host
    A(u) = smallest even a with p(a) <= u, then flip <=> |h| < A. Exact.

Per-core layout is feature-major ("transposed"): activations live as
[features(partitions), batch(free)], so batch stays on the free dim and no
on-device transposes are needed. Batch 16384 is sharded 2048/core.
"""

from contextlib import ExitStack

import numpy as np

import concourse.bass as bass  # noqa: F401
import concourse.tile as tile
from concourse import bacc, mybir
from concourse.bass_utils import run_bass_kernel_spmd

F32 = mybir.dt.float32
BF16 = mybir.dt.bfloat16
FP8 = mybir.dt.float8e4
ALU = mybir.AluOpType
ACTF = mybir.ActivationFunctionType
DR = mybir.MatmulPerfMode.DoubleRow

N_CORES = 8
B = 16384                 # full batch
BC = B // N_CORES         # batch per core
D_IN = 784                # layer-1 input features
D_H = 1024                # hidden features
D_OUT = 10                # output features
K1 = (D_IN + 127) // 128  # 7 k-chunks for layer 1 (6 full + 16 rows)
K1_LAST = D_IN - 128 * (K1 - 1)
KH = D_H // 128           # 8 k-chunks for hidden layers
OC = D_H // 128           # 8 output-feature chunks
NT = BC // 512            # moving tiles per psum row

# float32(0.5*exp(-(a*a)/50)) for a = 0,2,...,50, computed with jnp.exp on the
# same jax backend the reference uses (fallback if jax is unavailable here).
_PTABLE_BITS = [
    0x3F000000, 0x3EEC515A, 0x3EB9E4E3, 0x3E79375C, 0x3E0E5ACB, 0x3D8A9501,
    0x3CE5ED93, 0x3C2289CB, 0x3B43D285, 0x3A4909DD, 0x392FE09E, 0x38031DFC,
    0x36A696B8, 0x35345CD8, 0x33A6674D, 0x3202D2C5, 0x302F4A31, 0x2E4824C7,
    0x2C42BB52, 0x2A2173E9, 0x27E4229E, 0x258959AD, 0x230CEE5E, 0x207672F6,
    0x1DB79FE2, 0x1AE92B5E,
]


def _prob_table() -> np.ndarray:
    """p(a) for a = 0,2,...,50, bit-matching the reference's jnp.exp."""
    try:
        import jax.numpy as jnp

        a = np.arange(0, 51, 2, dtype=np.float32)
        p = np.asarray(0.5 * jnp.exp(-(jnp.asarray(a) * a) / (2.0 * 5.0**2)),
                       dtype=np.float32)
        if p.shape == (26,) and np.all(np.diff(p) < 0):
            return p
    except Exception:
        pass
    return np.array(_PTABLE_BITS, dtype=np.uint32).view(np.float32)


def _flip_thresholds(u: np.ndarray, ptable: np.ndarray) -> np.ndarray:
    """A(u): flip <=> |h| < A. A = 52 - 2 * #{a : p(a) <= u}."""
    tab = ptable[::-1].copy()  # ascending: p(50), p(48), ..., p(0)
    idx = np.searchsorted(tab, u, side="right")
    return (52 - 2 * idx).astype(np.float32)


def emit_noisy_sign_q(nc, pool, ps, a_ap, out_ap, sign_bias, sign_scale,
                      cmp_op, cg_bufs=2):
    """Write q = (flip - 0.5) * s = -(noisy sign)/2 into a +-0.5 fp8 panel.

    The true pre-activation h is an exact (half-)integer recoverable from the
    PSUM value ps via h = sign_scale' * ps; the caller passes sign_bias/scale
    so that s = Sign(sign_scale*ps + sign_bias) equals (h>0 ? +1 : -1)
    exactly (the bias offsets the Sign input off 0, where the reference maps
    h==0 to s=-1). flip = (|ps| < A) with the host pre-scaling A to match
    ps's encoding. Downstream matmuls of a q-panel produce -h/2, and the
    host doubles/negates the final output. All values stay exact.

    Engine split per chain: ScalarE computes the sign; DVE does the
    |h| multiply, compare, and fused (flip-0.5)*s panel write.
    """
    hb = ps.shape[-1]
    s = pool.tile([128, hb], F32, tag="s", bufs=cg_bufs)
    nc.scalar.activation(s[:], ps[:], ACTF.Sign, bias=sign_bias,
                         scale=sign_scale)                            # +-1
    d = pool.tile([128, hb], F32, tag="d", bufs=cg_bufs)
    # |ps| = ps * sgn(ps); s equals sgn(ps) except at ps==0 where s=-1 and
    # ps*s = 0 = |ps| anyway. Exact integer multiply.
    nc.vector.tensor_tensor(d[:], ps[:], s[:], op=ALU.mult)  # +-|h|-scaled
    f = pool.tile([128, hb], F32, tag="f", bufs=cg_bufs)
    nc.vector.tensor_tensor(f[:], d[:], a_ap, op=cmp_op)              # flip
    nc.vector.scalar_tensor_tensor(out_ap, f[:], -0.5, s[:],
                                   op0=ALU.add, op1=ALU.mult)         # q


def build_nc(repeat: int = 1, debug_taps: bool = False):
    """Build the per-core Bass program (same program on all 8 cores)."""
    nc = bacc.Bacc("TRN2", target_bir_lowering=False, debug=False,
                   num_devices=N_CORES)
    dbg = {}
    if debug_taps:
        for nm in ("db1", "db2", "db3"):
            dbg[nm] = nc.dram_tensor(nm, [D_H, BC], FP8,
                                     kind="ExternalOutput").ap()

    xt = nc.dram_tensor("xt", [D_IN, BC], F32, kind="ExternalInput").ap()
    a2 = nc.dram_tensor("a2", [D_H, BC], F32, kind="ExternalInput").ap()
    a3 = nc.dram_tensor("a3", [D_H, BC], F32, kind="ExternalInput").ap()
    w1 = nc.dram_tensor("w1", [D_IN, D_H], F32, kind="ExternalInput").ap()
    w2 = nc.dram_tensor("w2", [D_H, D_H], FP8, kind="ExternalInput").ap()
    w3 = nc.dram_tensor("w3", [D_H, D_H], FP8, kind="ExternalInput").ap()
    w4 = nc.dram_tensor("w4", [D_H, D_OUT], FP8, kind="ExternalInput").ap()
    c1 = nc.dram_tensor("c1", [128, OC], F32, kind="ExternalInput").ap()
    out = nc.dram_tensor("out", [D_OUT, BC], F32, kind="ExternalOutput").ap()

    with tile.TileContext(nc) as tc:
        with ExitStack() as ctx:
            consts = ctx.enter_context(tc.tile_pool(name="consts", bufs=1))
            panels = ctx.enter_context(tc.tile_pool(name="panels", bufs=1))

            # Layer-1-critical loads first (PE waits on these at startup).
            w1_t = consts.tile([128, K1, D_H], F32, tag="w1")
            c1_t = consts.tile([128, OC], F32, tag="c1")
            nh_t = consts.tile([128, 1], F32, tag="nh")
            nc.gpsimd.memset(nh_t[:], -0.5)
            nq_t = consts.tile([128, 1], F32, tag="nq")
            nc.gpsimd.memset(nq_t[:], -0.25)
            for k in range(K1 - 1):
                nc.sync.dma_start(w1_t[:, k], w1[k * 128:(k + 1) * 128, :])
            nc.sync.dma_start(w1_t[:K1_LAST, K1 - 1],
                              w1[128 * (K1 - 1):D_IN, :])
            nc.gpsimd.dma_start(c1_t[:], c1[:, :])

            # Hidden-layer weights: single 3D DMAs on the HWDGE (sync) path,
            # off the Pool engine's queue; needed only once layer 2 starts.
            w2_t = consts.tile([128, KH, D_H], FP8, tag="w2")
            w3_t = consts.tile([128, KH, D_H], FP8, tag="w3")
            w4_t = consts.tile([128, KH, D_OUT], FP8, tag="w4")
            nc.sync.dma_start(w2_t[:, :, :],
                              w2.rearrange("(k p) m -> p k m", p=128))
            nc.sync.dma_start(w3_t[:, :, :],
                              w3.rearrange("(k p) m -> p k m", p=128))
            nc.sync.dma_start(w4_t[:, :, :],
                              w4.rearrange("(k p) m -> p k m", p=128))

            # +-1 activation panels, feature-major fp8 (b3 is fused away).
            b1_t = panels.tile([128, KH, BC], FP8, tag="b1")
            b2_t = panels.tile([128, KH, BC], FP8, tag="b2")

            for _rep in range(repeat):
                # ---- Layer 1: fp32 matmul + threshold sign -> b1 ----
                with ExitStack() as l1s:
                    l1xp = l1s.enter_context(tc.tile_pool(name="l1xp", bufs=1))
                    l1psum = l1s.enter_context(
                        tc.tile_pool(name="l1psum", bufs=2, space="PSUM"))

                    xt_t = l1xp.tile([128, K1, BC], F32, tag="xt")
                    for n in range(NT):
                        nc.gpsimd.dma_start(
                            xt_t[:, 0, n * 512:(n + 1) * 512],
                            xt[0:128, n * 512:(n + 1) * 512])
                    for k in range(1, K1 - 1):
                        nc.gpsimd.dma_start(xt_t[:, k],
                                            xt[k * 128:(k + 1) * 128, :])
                    nc.gpsimd.dma_start(xt_t[:K1_LAST, K1 - 1],
                                        xt[128 * (K1 - 1):D_IN, :])

                    for o in range(OC):
                        ps = l1psum.tile([128, BC], F32, tag="mm")
                        for k in range(K1):
                            kk = K1_LAST if k == K1 - 1 else 128
                            lhsT = w1_t[:kk, k, o * 128:(o + 1) * 128]
                            for n in range(NT):
                                nc.tensor.matmul(
                                    ps[:, n * 512:(n + 1) * 512],
                                    lhsT,
                                    xt_t[:kk, k, n * 512:(n + 1) * 512],
                                    start=(k == 0),
                                    stop=(k == K1 - 1),
                                )
                        # b1 = sign(h1 - c1) in {-1, 0, +1}; c1 arrives
                        # negated so ACT computes Sign(h + (-c1)) in one op.
                        nc.scalar.activation(b1_t[:, o, :], ps[:], ACTF.Sign,
                                             bias=c1_t[:, o:o + 1])

                if debug_taps:
                    for o in range(OC):
                        nc.gpsimd.dma_start(dbg["db1"][o * 128:(o + 1) * 128, :],
                                            b1_t[:, o, :])
                # ---- Layer 2: fp8 DoubleRow matmul + exact noisy sign ----
                with ExitStack() as l2s:
                    lpsum = l2s.enter_context(
                        tc.tile_pool(name="l2psum", bufs=4, space="PSUM"))
                    ltmp = l2s.enter_context(tc.tile_pool(name="l2tmp", bufs=2))
                    lap = l2s.enter_context(tc.tile_pool(name="l2a", bufs=2))
                    HB = BC // 2
                    for o in range(OC):
                        a_t = lap.tile([128, BC], F32, tag="a")
                        nc.sync.dma_start(a_t[:], a2[o * 128:(o + 1) * 128, :])
                        for half in range(2):
                            hsl = slice(half * HB, (half + 1) * HB)
                            ps = lpsum.tile([128, HB], F32, tag="mm")
                            for kp in range(KH // 2):
                                lhsT = w2_t[:, 2 * kp:2 * kp + 2,
                                            o * 128:(o + 1) * 128]
                                for n in range(HB // 512):
                                    bsl = slice(half * HB + n * 512,
                                                half * HB + (n + 1) * 512)
                                    nc.tensor.matmul(
                                        ps[:, n * 512:(n + 1) * 512],
                                        lhsT,
                                        b1_t[:, 2 * kp:2 * kp + 2, bsl],
                                        start=(kp == 0),
                                        stop=(kp == KH // 2 - 1),
                                        perf_mode=DR,
                                    )
                            emit_noisy_sign_q(nc, ltmp, ps, a_t[:, hsl],
                                              b2_t[:, o, hsl], nh_t[:], 1.0,
                                              ALU.is_lt)

                if debug_taps:
                    for o in range(OC):
                        nc.gpsimd.dma_start(dbg["db2"][o * 128:(o + 1) * 128, :],
                                            b2_t[:, o, :])
                # ---- Layer 3 -> b3 panel ----
                b3_t = panels.tile([128, KH, BC], FP8, tag="b3")
                with ExitStack() as l3s:
                    lpsum = l3s.enter_context(
                        tc.tile_pool(name="l3psum", bufs=4, space="PSUM"))
                    ltmp = l3s.enter_context(tc.tile_pool(name="l3tmp", bufs=2))
                    lap = l3s.enter_context(tc.tile_pool(name="l3a", bufs=2))

                    HB = BC // 2
                    for o in range(OC):
                        a_t = lap.tile([128, BC], F32, tag="a")
                        nc.sync.dma_start(a_t[:], a3[o * 128:(o + 1) * 128, :])
                        for half in range(2):
                            hsl = slice(half * HB, (half + 1) * HB)
                            ps = lpsum.tile([128, HB], F32, tag="mm")
                            for kp in range(KH // 2):
                                lhsT = w3_t[:, 2 * kp:2 * kp + 2,
                                            o * 128:(o + 1) * 128]
                                for n in range(HB // 512):
                                    bsl = slice(half * HB + n * 512,
                                                half * HB + (n + 1) * 512)
                                    nc.tensor.matmul(
                                        ps[:, n * 512:(n + 1) * 512],
                                        lhsT,
                                        b2_t[:, 2 * kp:2 * kp + 2, bsl],
                                        start=(kp == 0),
                                        stop=(kp == KH // 2 - 1),
                                        perf_mode=DR,
                                    )
                            emit_noisy_sign_q(nc, ltmp, ps, a_t[:, hsl],
                                              b3_t[:, o, hsl], nq_t[:], -1.0,
                                              ALU.is_gt)

                if debug_taps:
                    for o in range(OC):
                        nc.gpsimd.dma_start(dbg["db3"][o * 128:(o + 1) * 128, :],
                                            b3_t[:, o, :])
                # ---- Layer 4: out = b3 @ sign(W4).T ----
                with ExitStack() as l4s:
                    l4psum = l4s.enter_context(
                        tc.tile_pool(name="l4psum", bufs=1, space="PSUM"))
                    l4out = l4s.enter_context(tc.tile_pool(name="l4out", bufs=1))
                    ops = l4psum.tile([D_OUT, BC], F32, tag="out")
                    for k in range(KH):
                        for n in range(NT):
                            nc.tensor.matmul(
                                ops[:, n * 512:(n + 1) * 512],
                                w4_t[:, k, :],
                                b3_t[:, k, n * 512:(n + 1) * 512],
                                start=(k == 0),
                                stop=(k == KH - 1),
                            )
                    ot = l4out.tile([D_OUT, BC], F32, tag="ot")
                    nc.scalar.activation(ot[:], ops[:], ACTF.Copy)
                    nc.gpsimd.dma_start(out[:, :], ot[:])

    nc.compile()
    return nc


_NC_CACHE: dict[int, object] = {}


def _get_nc(repeat: int = 1):
    if repeat not in _NC_CACHE:
        _NC_CACHE[repeat] = build_nc(repeat)
    return _NC_CACHE[repeat]


def make_in_maps(x, u2, u3, W1, W2, W3, W4, **_unused):
    """Host preprocessing -> per-core input dicts."""
    fp8_np = mybir.dt.np(FP8)

    x = np.ascontiguousarray(np.asarray(x, dtype=np.float32))
    W1b = np.sign(np.asarray(W1, dtype=np.float32))
    # mean(h1, axis=0) = sign(W1) @ mean(x, axis=0), in float64.
    mu1 = (W1b.astype(np.float64) @ x.mean(axis=0, dtype=np.float64)).astype(
        np.float32)
    # negated: the device computes Sign(h + bias) with bias = -mu1
    c1 = np.ascontiguousarray((-mu1).reshape(OC, 128).T)  # [128, OC]

    pt = _prob_table()
    a2f = _flip_thresholds(np.asarray(u2), pt)
    # layer-3 PSUM holds -h3/2 and d = ps*sgn(h) = -|h3|/2, so the flip test
    # becomes d > -A/2 (A even -> A/2 exact; negation exact)
    a3f = _flip_thresholds(np.asarray(u3), pt) * np.float32(-0.5)

    xt = np.ascontiguousarray(x.T)                       # [784, B]
    a2t = np.ascontiguousarray(a2f.T)                    # [1024, B]
    a3t = np.ascontiguousarray(a3f.T)
    w1t = np.ascontiguousarray(W1b.T)                    # [784, 1024] fp32
    w2t = np.ascontiguousarray(
        np.sign(np.asarray(W2, np.float32)).T).astype(fp8_np)
    w3t = np.ascontiguousarray(
        np.sign(np.asarray(W3, np.float32)).T).astype(fp8_np)
    w4t = np.ascontiguousarray(
        np.sign(np.asarray(W4, np.float32)).T).astype(fp8_np)

    in_maps = []
    for c in range(N_CORES):
        sl = slice(c * BC, (c + 1) * BC)
        in_maps.append({
            "xt": np.ascontiguousarray(xt[:, sl]),
            "a2": np.ascontiguousarray(a2t[:, sl]),
            "a3": np.ascontiguousarray(a3t[:, sl]),
            "w1": w1t,
            "w2": w2t,
            "w3": w3t,
            "w4": w4t,
            "c1": c1,
        })
    return in_maps


def kernel(x, u2, u3, W1, W2, W3, W4,
           g1=None, b1=None, g2=None, b2=None, g3=None, b3=None):
    for g in (g1, g2, g3):
        assert g is None or np.all(np.asarray(g) > 0), "kernel assumes g > 0"
    for b in (b1, b2, b3):
        assert b is None or np.all(np.asarray(b) == 0), "kernel assumes b == 0"

    nc = _get_nc(repeat=1)
    in_maps = make_in_maps(x, u2, u3, W1, W2, W3, W4)
    res = run_bass_kernel_spmd(nc, in_maps, core_ids=list(range(N_CORES)))

    # device computes -out/2 (q-encoded panels); doubling is exact in fp32
    out = np.empty((B, D_OUT), dtype=np.float32)
    for c in range(N_CORES):
        out[c * BC:(c + 1) * BC, :] = np.float32(-2.0) * res.results[c]["out"].T
    return out
